# revision 34
# baseline (speedup 1.0000x reference)
"""Trainium2 Bass kernel for nn_AttentionAggregator.

Computation (per side, users/items symmetric):
    cu  = concat(gather(review_vecs, adj_r), gather(sec_vecs, adj_s))   # [6000, 1024]
    att = softmax(keys @ keys.T / 8) @ cu                               # [6000, 1024]
    out = relu(att @ W)                                                 # [6000, 1024]

Sharding: 8 cores run the same program (SPMD). Cores 0-3 take the user side
(1500 query rows each), cores 4-7 the item side. Keys, gather sources,
adjacency and weights are replicated; only the query slice differs.

On-device per core:
  - gather cu tile-by-tile from DRAM via ONE indirect DMA per 128-key tile
    (16 neighbor indices per key row -> 2048 descriptors per instruction;
    the r/s sources are concatenated on host so one instruction covers both)
  - scoresT[k,q] = keys @ q.T via PE in bf16 (contraction over D=64,
    zero-padded to 128)
  - E = exp(scoresT/8) on ScalarE directly PSUM->SBUF (no max-subtraction
    needed: |scores/8| <= ~5 in fp32)
  - O = E.T-weighted sum of cu, accumulated on PE in PSUM over chunks of
    k-tiles, then folded into an SBUF fp32 accumulator by DVE
  - rowsums r = E.T @ ones accumulated in a persistent PSUM bank
  - out = relu(O @ W) * (1/r), with the 1/r per-partition scale fused into
    the final ReLU PSUM->SBUF copy (valid since r > 0)

Column layout of the gathered cu is [review slots 0-7 | sec slots 0-7]
(instead of the reference's interleaved layout); the host permutes W's rows
to match, so results are identical.
"""

import os
import sys

import ml_dtypes
import numpy as np

for _p in ("/opt/trn_rl_repo", "/root/.axon_site/_ro/trn_rl_repo"):
    if os.path.isdir(_p) and _p not in sys.path:
        sys.path.append(_p)

import concourse.bass as bass  # noqa: E402
import concourse.mybir as mybir  # noqa: E402
import concourse.tile as tile  # noqa: E402
from concourse import bacc  # noqa: E402
from concourse.bass_utils import run_bass_kernel_spmd  # noqa: E402
from concourse.masks import make_identity  # noqa: E402

P = 128
D = 64
DP = 128           # gather-source row pitch (bf16 rows padded 64 -> 128 so
                   # each row is 256 B, the dma_gather stride granularity)
NK = 6000          # keys per side
NKP = 6144         # padded to 48 full k-tiles
KT = NKP // P      # 48
KT_CALC = 47       # k-tiles that carry real keys (kt 47 is all padding)
QOUT = 1500        # query rows per core (6000 / 4 cores per side)
QP = 1536          # padded to 12 full q-subtiles
NQS = QP // P      # 12
HID = 1024
NR = 30000         # review_vecs rows
NS = 6000          # secondary source rows
CHUNK_SIZES = tuple(int(x) for x in os.environ.get("K_CHUNKS", "2,4,6,6,6,6,6,6,6").split(","))
assert sum(CHUNK_SIZES) == 48
CHUNK_STARTS = tuple(int(np.cumsum((0,) + CHUNK_SIZES)[i]) for i in range(len(CHUNK_SIZES)))
F32 = mybir.dt.float32
BF16 = mybir.dt.bfloat16
I32 = mybir.dt.int32
I16 = mybir.dt.int16

AF = mybir.ActivationFunctionType


def _emit_body(nc, tc, ctx_pools, tensors):
    """Emit one full pass of the kernel body inside an open TileContext."""
    from contextlib import ExitStack

    keysT, qvT, adj_r, adj_s, src_r, src_s, w, ebias, out = tensors
    const, psum, psum_b, psum_r = ctx_pools

    # ---- persistent tiles -------------------------------------------------
    identity = const.tile([P, P], F32, tag="identity")
    make_identity(nc, identity[:])
    ones = const.tile([P, 1], BF16, tag="ones")
    nc.gpsimd.memset(ones[:], 1.0)

    # load order matters for the pipeline fill: queries + first key columns
    # first (gate the first scores), then adjacency (gates the gathers).
    # 64-partition tiles: the matmul contraction dim is just D=64, no
    # zero-padding to 128 needed.
    qvT_sb = const.tile([D, QP], BF16, tag="qvT")
    nc.sync.dma_start(qvT_sb[:, 0:512], qvT[:, 0:512])
    nc.sync.dma_start(qvT_sb[:, 512:], qvT[:, 512:])

    vecsT = const.tile([D, NKP], BF16, tag="vecsT")
    nc.sync.dma_start(vecsT[:, :NKP // 8], keysT[:, :NKP // 8])

    adj_r_sb = const.tile([P, KT, 64], I16, tag="adjr")
    nc.sync.dma_start(adj_r_sb[:], adj_r[:, :, :])
    adj_s_sb = const.tile([P, KT, 64], I16, tag="adjs")
    nc.sync.dma_start(adj_s_sb[:], adj_s[:, :, :])

    nc.sync.dma_start(vecsT[:, NKP // 8:NKP // 2], keysT[:, NKP // 8:NKP // 2])
    nc.sync.dma_start(vecsT[:, NKP // 2:], keysT[:, NKP // 2:])

    ebias_sb = const.tile([P, 1], F32, tag="ebias")
    nc.sync.dma_start(ebias_sb[:], ebias[:, :])

    o_acc = const.tile([P, NQS, HID], F32, tag="oacc")
    r_acc = const.tile([P, NQS], F32, tag="racc")
    rinv = const.tile([P, NQS], F32, tag="rinv")

    w_sb = const.tile([P, HID // P, HID], BF16, tag="w")
    nc.sync.dma_start(w_sb[:], w[:, :, :])

    chunks = [list(range(st, min(st + cs, KT_CALC)))
              for st, cs in zip(CHUNK_STARTS, CHUNK_SIZES)]
    chunks = [c for c in chunks if c]

    with ExitStack() as ctx:
        e_pool = ctx.enter_context(tc.tile_pool(name="e_pool", bufs=12))
        g_pool = ctx.enter_context(tc.tile_pool(name="g_pool", bufs=12))
        ot_pool = ctx.enter_context(tc.tile_pool(name="ot_pool", bufs=4))
        ob_pool = ctx.enter_context(tc.tile_pool(name="ob_pool", bufs=3))

        e_tiles = {}
        g_tiles = {}

        def emit_gather(chunk):
            # two dma_gathers per k-tile (8 review + 8 secondary neighbor
            # rows per key row): 1024 indices each, one 256 B padded source
            # row per index. Result tiles are [128, 8 slots, 128] with the
            # payload in cols 0:64 of each slot (the numerator matmul reads
            # the strided [.., 0:64] view directly).
            for kt in chunk:
                gr = g_pool.tile([P, 8, DP], BF16, tag="gr")
                nc.gpsimd.dma_gather(
                    out_ap=gr[:],
                    in_ap=src_r[:],
                    idxs_ap=adj_r_sb[:, kt, :],
                    num_idxs=1024,
                    num_idxs_reg=1024,
                    elem_size=DP,
                )
                gs = g_pool.tile([P, 8, DP], BF16, tag="gs")
                nc.gpsimd.dma_gather(
                    out_ap=gs[:],
                    in_ap=src_s[:],
                    idxs_ap=adj_s_sb[:, kt, :],
                    num_idxs=1024,
                    num_idxs_reg=1024,
                    elem_size=DP,
                )
                g_tiles[kt] = (gr, gs)

        def score_items(chunk):
            """One (matmul+exp) emission per item; consumed a few per
            numerator iteration so the exps overlap the previous chunk's
            numerator instead of serializing before this chunk's."""
            for kt in chunk:
                e = e_pool.tile([P, QP], BF16, tag="e")
                e_tiles[kt] = e
                for i in range(QP // 512):
                    yield kt, e, i

        def emit_score(item):
            kt, e, i = item
            lhsT = vecsT[:, kt * P:(kt + 1) * P]  # [64, 128]: K=64 contraction
            s_ps = psum.tile([P, 512], F32, tag="ps")
            nc.tensor.matmul(
                s_ps[:], lhsT, qvT_sb[:, i * 512:(i + 1) * 512],
                start=True, stop=True,
            )
            # padded key rows (6000..6015) get bias -1e30 so exp()
            # forces their attention weight to exactly zero
            bias = ebias_sb[:, 0:1] if kt == KT_CALC - 1 else 0.0
            nc.scalar.activation(
                e[:, i * 512:(i + 1) * 512], s_ps[:], AF.Exp,
                bias=bias, scale=0.125,
            )

        def emit_phase_b(j):
            """Transpose o_acc[:, j], project through W, relu*1/r, store."""
            ots = []
            for half in range(2):
                tp = psum_b.tile([P, 512], F32, tag="pb")
                for t in range(4):
                    nc.tensor.transpose(
                        tp[:, t * P:(t + 1) * P],
                        o_acc[:, j, (half * 4 + t) * P:(half * 4 + t + 1) * P],
                        identity[:],
                    )
                ot = ot_pool.tile([P, 512], BF16, tag="ot")
                nc.vector.tensor_copy(ot[:], tp[:])
                ots.append(ot)
            for h in range(HID // 512):
                pf = psum_b.tile([P, 512], F32, tag="pb")
                for t in range(HID // P):
                    nc.tensor.matmul(
                        pf[:], ots[t // 4][:, (t % 4) * P:(t % 4 + 1) * P],
                        w_sb[:, t, h * 512:(h + 1) * 512],
                        start=(t == 0), stop=(t == HID // P - 1),
                    )
                ob = ob_pool.tile([P, 512], F32, tag="ob")
                nc.scalar.activation(ob[:], pf[:], AF.Relu, scale=rinv[:, j:j + 1])
                rows = min(P, QOUT - j * P)
                if rows > 0:
                    nc.sync.dma_start(
                        out[j * P:j * P + rows, h * 512:(h + 1) * 512], ob[:rows, :],
                    )

        emit_gather(chunks[0])
        for item in score_items(chunks[0]):
            emit_score(item)

        for ci, chunk in enumerate(chunks):
            first_chunk = ci == 0
            last_chunk = ci == len(chunks) - 1
            if not last_chunk:
                emit_gather(chunks[ci + 1])
                next_scores = score_items(chunks[ci + 1])
                per_j = -(-3 * len(chunks[ci + 1]) // NQS)
            else:
                next_scores = iter(())
                per_j = 0

            r_ps = psum_r.tile([P, NQS], F32, tag="rps")
            for j in range(NQS):
                p0 = psum.tile([P, 512], F32, tag="ps")
                p1 = psum.tile([P, 512], F32, tag="ps")
                for i, kt in enumerate(chunk):
                    lhsT = e_tiles[kt][:, j * P:(j + 1) * P]
                    first = i == 0
                    last = i == len(chunk) - 1
                    gr, gs = g_tiles[kt]
                    nc.tensor.matmul(p0[:], lhsT, gr[:, :, 0:D],
                                     start=first, stop=last)
                    nc.tensor.matmul(p1[:], lhsT, gs[:, :, 0:D],
                                     start=first, stop=last)
                    nc.tensor.matmul(r_ps[:, j:j + 1], lhsT, ones[:],
                                     start=first, stop=last)
                if first_chunk:
                    nc.vector.tensor_copy(o_acc[:, j, 0:512], p0[:])
                    nc.vector.tensor_copy(o_acc[:, j, 512:1024], p1[:])
                else:
                    nc.vector.tensor_add(o_acc[:, j, 0:512], o_acc[:, j, 0:512], p0[:])
                    nc.vector.tensor_add(o_acc[:, j, 512:1024], o_acc[:, j, 512:1024], p1[:])
                for _ in range(per_j):
                    item = next(next_scores, None)
                    if item is not None:
                        emit_score(item)
                if last_chunk:
                    # final fold for this q-block: finish rowsum, invert,
                    # then phase B of the PREVIOUS block (so PE never waits
                    # on this block's DVE fold)
                    nc.vector.tensor_add(r_acc[:, j:j + 1], r_acc[:, j:j + 1],
                                         r_ps[:, j:j + 1])
                    nc.vector.reciprocal(rinv[:, j:j + 1], r_acc[:, j:j + 1])
                    if j > 0:
                        emit_phase_b(j - 1)
            for item in next_scores:
                emit_score(item)
            if first_chunk:
                nc.vector.tensor_copy(r_acc[:], r_ps[:])
            elif not last_chunk:
                nc.vector.tensor_add(r_acc[:], r_acc[:], r_ps[:])

        emit_phase_b(NQS - 1)


def build_program(repeat: int = 0, scratch: int | None = 32768):
    """Build + compile the SPMD program. repeat>0 wraps the body in a
    device-side For loop (for timing) and is not used for grading."""
    from contextlib import ExitStack

    kw = {} if scratch is None else dict(dynamic_dma_scratch_size=scratch)
    nc = bacc.Bacc("TRN2", target_bir_lowering=False, debug=False, num_devices=8, **kw)

    keysT = nc.dram_tensor("keysT", [D, NKP], BF16, kind="ExternalInput")
    qvT = nc.dram_tensor("qvT", [D, QP], BF16, kind="ExternalInput")
    adj_r = nc.dram_tensor("adj_r", [P, KT, 64], I16, kind="ExternalInput")
    adj_s = nc.dram_tensor("adj_s", [P, KT, 64], I16, kind="ExternalInput")
    src_r = nc.dram_tensor("src_r", [NR, DP], BF16, kind="ExternalInput")
    src_s = nc.dram_tensor("src_s", [NS, DP], BF16, kind="ExternalInput")
    w = nc.dram_tensor("w", [P, HID // P, HID], BF16, kind="ExternalInput")
    ebias = nc.dram_tensor("ebias", [P, 1], F32, kind="ExternalInput")
    out = nc.dram_tensor("out", [QOUT, HID], F32, kind="ExternalOutput")

    tensors = (keysT, qvT, adj_r, adj_s, src_r, src_s, w, ebias, out)

    with tile.TileContext(nc) as tc, ExitStack() as ctx:
        const = ctx.enter_context(tc.tile_pool(name="const", bufs=1))
        psum = ctx.enter_context(tc.tile_pool(name="psum", bufs=5, space="PSUM"))
        psum_b = ctx.enter_context(tc.tile_pool(name="psum_b", bufs=2, space="PSUM"))
        psum_r = ctx.enter_context(tc.tile_pool(name="psum_r", bufs=1, space="PSUM"))
        pools = (const, psum, psum_b, psum_r)
        for _ in range(max(repeat, 1)):
            _emit_body(nc, tc, pools, tensors)

    nc.compile()
    return nc


def _permute_w(w_full: np.ndarray) -> np.ndarray:
    """Reference cu columns are slot-interleaved [r0 i0 r1 i1 ...]; the kernel
    gathers [r0..r7 | i0..i7]. Permute W rows to match, then pre-tile to
    [128, 8, 1024] for the on-device layout."""
    wr = w_full.reshape(8, 2, D, HID)
    w_perm = np.concatenate(
        [wr[:, 0].reshape(8 * D, HID), wr[:, 1].reshape(8 * D, HID)], axis=0,
    )
    return np.ascontiguousarray(
        w_perm.reshape(HID // P, P, HID).transpose(1, 0, 2),
    )


def _wrap_adj(adj: np.ndarray) -> np.ndarray:
    """[6000, 8] -> [128, KT, 64] int16 in dma_gather index layout: per
    k-tile, flat index i = slot*128 + row (so gathered row i lands at
    out[i%128, i//128]), wrapped into 16 partitions (entry n at
    [n%16, n//16]) and replicated 8x to fill 128 partitions. Padded key
    rows index row 0 (their attention weight is forced to zero by ebias)."""
    a = np.zeros((NKP, 8), dtype=np.int64)
    a[:NK] = adj
    tiles = a.reshape(KT, P, 8)                                # [kt, p, c]
    idx_flat = tiles.transpose(0, 2, 1).reshape(KT, 1024)      # i = c*128+p
    idx16 = idx_flat.reshape(KT, 64, 16).transpose(0, 2, 1)    # [kt, 16, 64]
    full = np.tile(idx16, (1, 8, 1))                           # [kt, 128, 64]
    return np.ascontiguousarray(full.transpose(1, 0, 2)).astype(np.int16)


def _host_inputs(review_vecs, user_vecs, item_vecs, user_weights, item_weights,
                 user_review_adj, user_item_adj, item_review_adj, item_user_adj):
    review_vecs = np.asarray(review_vecs, dtype=np.float32)
    user_vecs = np.asarray(user_vecs, dtype=np.float32)
    item_vecs = np.asarray(item_vecs, dtype=np.float32)

    def _pad_src(v: np.ndarray) -> np.ndarray:
        out = np.zeros((v.shape[0], DP), dtype=ml_dtypes.bfloat16)
        out[:, :D] = v.astype(ml_dtypes.bfloat16)
        return out

    src_r_pad = _pad_src(review_vecs)
    sides = {}
    for side, keys, adj_r, adj_s, src_s, w_full in (
        ("user", user_vecs, user_review_adj, user_item_adj, item_vecs, user_weights),
        ("item", item_vecs, item_review_adj, item_user_adj, user_vecs, item_weights),
    ):
        keysT = np.zeros((D, NKP), dtype=ml_dtypes.bfloat16)
        keysT[:, :NK] = keys.T.astype(ml_dtypes.bfloat16)
        sides[side] = dict(
            keysT=keysT,
            adj_r=_wrap_adj(np.asarray(adj_r, dtype=np.int64)),
            adj_s=_wrap_adj(np.asarray(adj_s, dtype=np.int64)),
            src_s=_pad_src(np.asarray(src_s, dtype=np.float32)),
            w=_permute_w(np.asarray(w_full, dtype=np.float32)),
            keys=keys,
        )

    ebias = np.zeros((P, 1), dtype=np.float32)
    ebias[NK - (KT_CALC - 1) * P:] = -1e30

    in_maps = []
    for c in range(8):
        s = sides["user" if c < 4 else "item"]
        b = c % 4
        qv = s["keys"][b * QOUT:(b + 1) * QOUT]  # [1500, 64]
        qvT = np.empty((D, QP), dtype=np.float32)
        qvT[:, :QOUT] = qv.T
        qvT[:, QOUT:] = qv.T[:, :QP - QOUT]  # pad with real vectors (finite rowsums)
        in_maps.append(dict(
            keysT=s["keysT"], qvT=np.ascontiguousarray(qvT.astype(ml_dtypes.bfloat16)),
            adj_r=s["adj_r"], adj_s=s["adj_s"],
            src_r=src_r_pad, src_s=s["src_s"],
            w=s["w"].astype(ml_dtypes.bfloat16), ebias=ebias,
        ))
    return in_maps


_NC_CACHE = None


def kernel(**inputs):
    global _NC_CACHE
    if _NC_CACHE is None:
        _NC_CACHE = build_program()
    nc = _NC_CACHE
    in_maps = _host_inputs(**inputs)
    res = run_bass_kernel_spmd(nc, in_maps, core_ids=list(range(8)))
    outs = [res.results[c]["out"] for c in range(8)]
    user_output = np.concatenate(outs[0:4], axis=0)
    item_output = np.concatenate(outs[4:8], axis=0)
    return user_output, item_output


# revision 35
# speedup vs baseline: 1.0777x; 1.0777x over previous
"""Trainium2 Bass kernel for nn_AttentionAggregator.

Computation (per side, users/items symmetric):
    cu  = concat(gather(review_vecs, adj_r), gather(sec_vecs, adj_s))   # [6000, 1024]
    att = softmax(keys @ keys.T / 8) @ cu                               # [6000, 1024]
    out = relu(att @ W)                                                 # [6000, 1024]

Sharding: 8 cores run the same program (SPMD). Cores 0-3 take the user side
(1500 query rows each), cores 4-7 the item side. Keys, gather sources,
adjacency and weights are replicated; only the query slice differs.

On-device per core:
  - gather cu tile-by-tile from DRAM via ONE indirect DMA per 128-key tile
    (16 neighbor indices per key row -> 2048 descriptors per instruction;
    the r/s sources are concatenated on host so one instruction covers both)
  - scoresT[k,q] = keys @ q.T via PE in bf16 (contraction over D=64,
    zero-padded to 128)
  - E = exp(scoresT/8) on ScalarE directly PSUM->SBUF (no max-subtraction
    needed: |scores/8| <= ~5 in fp32)
  - O = E.T-weighted sum of cu, accumulated on PE in PSUM over chunks of
    k-tiles, then folded into an SBUF fp32 accumulator by DVE
  - rowsums r = E.T @ ones accumulated in a persistent PSUM bank
  - out = relu(O @ W) * (1/r), with the 1/r per-partition scale fused into
    the final ReLU PSUM->SBUF copy (valid since r > 0)

Column layout of the gathered cu is [review slots 0-7 | sec slots 0-7]
(instead of the reference's interleaved layout); the host permutes W's rows
to match, so results are identical.
"""

import os
import sys

import ml_dtypes
import numpy as np

for _p in ("/opt/trn_rl_repo", "/root/.axon_site/_ro/trn_rl_repo"):
    if os.path.isdir(_p) and _p not in sys.path:
        sys.path.append(_p)

import concourse.bass as bass  # noqa: E402
import concourse.mybir as mybir  # noqa: E402
import concourse.tile as tile  # noqa: E402
from concourse import bacc  # noqa: E402
from concourse.bass_utils import run_bass_kernel_spmd  # noqa: E402
from concourse.masks import make_identity  # noqa: E402

P = 128
D = 64
DP = 128           # gather-source row pitch (bf16 rows padded 64 -> 128 so
                   # each row is 256 B, the dma_gather stride granularity)
NK = 6000          # keys per side
NKP = 6144         # padded to 48 full k-tiles
KT = NKP // P      # 48
KT_CALC = 47       # k-tiles that carry real keys (kt 47 is all padding)
QOUT = 1500        # query rows per core (6000 / 4 cores per side)
QP = 1536          # padded to 12 full q-subtiles
NQS = QP // P      # 12
HID = 1024
NR = 30000         # review_vecs rows
NS = 6000          # secondary source rows
CHUNK_SIZES = tuple(int(x) for x in os.environ.get("K_CHUNKS", "2,4,6,6,6,6,6,6,6").split(","))
assert sum(CHUNK_SIZES) == 48
CHUNK_STARTS = tuple(int(np.cumsum((0,) + CHUNK_SIZES)[i]) for i in range(len(CHUNK_SIZES)))
F32 = mybir.dt.float32
BF16 = mybir.dt.bfloat16
I32 = mybir.dt.int32
I16 = mybir.dt.int16

AF = mybir.ActivationFunctionType


def _emit_body(nc, tc, ctx_pools, tensors):
    """Emit one full pass of the kernel body inside an open TileContext."""
    from contextlib import ExitStack

    keysT, qvT, adj_r, adj_s, src_r, src_s, w, ebias, out = tensors
    const, psum, psum_b, psum_r = ctx_pools

    # ---- persistent tiles -------------------------------------------------
    identity = const.tile([P, P], F32, tag="identity")
    make_identity(nc, identity[:])
    ones = const.tile([P, 1], BF16, tag="ones")
    nc.gpsimd.memset(ones[:], 1.0)

    # load order matters for the pipeline fill: queries + first key columns
    # first (gate the first scores), then adjacency (gates the gathers).
    # 64-partition tiles: the matmul contraction dim is just D=64, no
    # zero-padding to 128 needed.
    qvT_sb = const.tile([D, QP], BF16, tag="qvT")
    nc.sync.dma_start(qvT_sb[:, 0:512], qvT[:, 0:512])
    nc.sync.dma_start(qvT_sb[:, 512:], qvT[:, 512:])

    vecsT = const.tile([D, NKP], BF16, tag="vecsT")
    nc.sync.dma_start(vecsT[:, :NKP // 8], keysT[:, :NKP // 8])

    adj_r_sb = const.tile([P, KT, 64], I16, tag="adjr")
    nc.sync.dma_start(adj_r_sb[:], adj_r[:, :, :])
    adj_s_sb = const.tile([P, KT, 64], I16, tag="adjs")
    nc.sync.dma_start(adj_s_sb[:], adj_s[:, :, :])

    nc.sync.dma_start(vecsT[:, NKP // 8:NKP // 2], keysT[:, NKP // 8:NKP // 2])
    nc.sync.dma_start(vecsT[:, NKP // 2:], keysT[:, NKP // 2:])

    ebias_sb = const.tile([P, 1], F32, tag="ebias")
    nc.sync.dma_start(ebias_sb[:], ebias[:, :])

    o_acc = const.tile([P, NQS, HID], F32, tag="oacc")
    r_acc = const.tile([P, NQS], F32, tag="racc")
    rinv = const.tile([P, NQS], F32, tag="rinv")

    w_sb = const.tile([P, HID // P, HID], BF16, tag="w")
    nc.sync.dma_start(w_sb[:], w[:, :, :])

    chunks = [list(range(st, min(st + cs, KT_CALC)))
              for st, cs in zip(CHUNK_STARTS, CHUNK_SIZES)]
    chunks = [c for c in chunks if c]

    with ExitStack() as ctx:
        e_pool = ctx.enter_context(tc.tile_pool(name="e_pool", bufs=12))
        g_pool = ctx.enter_context(tc.tile_pool(name="g_pool", bufs=12))
        ot_pool = ctx.enter_context(tc.tile_pool(name="ot_pool", bufs=4))
        ob_pool = ctx.enter_context(tc.tile_pool(name="ob_pool", bufs=3))

        e_tiles = {}
        g_tiles = {}

        def emit_gather(chunk):
            # two dma_gathers per k-tile (8 review + 8 secondary neighbor
            # rows per key row): 1024 indices each, one 256 B padded source
            # row per index. Result tiles are [128, 8 slots, 128] with the
            # payload in cols 0:64 of each slot (the numerator matmul reads
            # the strided [.., 0:64] view directly).
            for kt in chunk:
                gr = g_pool.tile([P, 8, DP], BF16, tag="gr")
                nc.gpsimd.dma_gather(
                    out_ap=gr[:],
                    in_ap=src_r[:],
                    idxs_ap=adj_r_sb[:, kt, :],
                    num_idxs=1024,
                    num_idxs_reg=1024,
                    elem_size=DP,
                )
                gs = g_pool.tile([P, 8, DP], BF16, tag="gs")
                nc.gpsimd.dma_gather(
                    out_ap=gs[:],
                    in_ap=src_s[:],
                    idxs_ap=adj_s_sb[:, kt, :],
                    num_idxs=1024,
                    num_idxs_reg=1024,
                    elem_size=DP,
                )
                g_tiles[kt] = (gr, gs)

        def score_items(chunk):
            """One (matmul+exp) emission per item; consumed a few per
            numerator iteration so the exps overlap the previous chunk's
            numerator instead of serializing before this chunk's."""
            for kt in chunk:
                e = e_pool.tile([P, QP], BF16, tag="e")
                e_tiles[kt] = e
                for i in range(QP // 512):
                    yield kt, e, i

        def emit_score(item):
            kt, e, i = item
            lhsT = vecsT[:, kt * P:(kt + 1) * P]  # [64, 128]: K=64 contraction
            s_ps = psum.tile([P, 512], F32, tag="ps")
            nc.tensor.matmul(
                s_ps[:], lhsT, qvT_sb[:, i * 512:(i + 1) * 512],
                start=True, stop=True,
            )
            # padded key rows (6000..6015) get bias -1e30 so exp()
            # forces their attention weight to exactly zero
            bias = ebias_sb[:, 0:1] if kt == KT_CALC - 1 else 0.0
            nc.scalar.activation(
                e[:, i * 512:(i + 1) * 512], s_ps[:], AF.Exp,
                bias=bias, scale=0.125,
            )

        def emit_phase_b(j):
            """Transpose o_acc[:, j], project through W, relu*1/r, store."""
            ots = []
            for half in range(2):
                tp = psum_b.tile([P, 512], F32, tag="pb")
                for t in range(4):
                    nc.tensor.transpose(
                        tp[:, t * P:(t + 1) * P],
                        o_acc[:, j, (half * 4 + t) * P:(half * 4 + t + 1) * P],
                        identity[:],
                    )
                ot = ot_pool.tile([P, 512], BF16, tag="ot")
                nc.vector.tensor_copy(ot[:], tp[:])
                ots.append(ot)
            for h in range(HID // 512):
                pf = psum_b.tile([P, 512], F32, tag="pb")
                for t in range(HID // P):
                    nc.tensor.matmul(
                        pf[:], ots[t // 4][:, (t % 4) * P:(t % 4 + 1) * P],
                        w_sb[:, t, h * 512:(h + 1) * 512],
                        start=(t == 0), stop=(t == HID // P - 1),
                    )
                ob = ob_pool.tile([P, 512], F32, tag="ob")
                nc.scalar.activation(ob[:], pf[:], AF.Relu, scale=rinv[:, j:j + 1])
                rows = min(P, QOUT - j * P)
                if rows > 0:
                    nc.sync.dma_start(
                        out[j * P:j * P + rows, h * 512:(h + 1) * 512], ob[:rows, :],
                    )

        emit_gather(chunks[0])
        for item in score_items(chunks[0]):
            emit_score(item)

        for ci, chunk in enumerate(chunks):
            first_chunk = ci == 0
            last_chunk = ci == len(chunks) - 1
            if not last_chunk:
                emit_gather(chunks[ci + 1])
                next_scores = score_items(chunks[ci + 1])
                per_j = -(-3 * len(chunks[ci + 1]) // NQS)
            else:
                next_scores = iter(())
                per_j = 0

            r_ps = psum_r.tile([P, NQS], F32, tag="rps")
            for j in range(NQS):
                p0 = psum.tile([P, 512], F32, tag="ps")
                p1 = psum.tile([P, 512], F32, tag="ps")
                for i, kt in enumerate(chunk):
                    lhsT = e_tiles[kt][:, j * P:(j + 1) * P]
                    first = i == 0
                    last = i == len(chunk) - 1
                    gr, gs = g_tiles[kt]
                    nc.tensor.matmul(p0[:], lhsT, gr[:, :, 0:D],
                                     start=first, stop=last)
                    nc.tensor.matmul(p1[:], lhsT, gs[:, :, 0:D],
                                     start=first, stop=last)
                    nc.tensor.matmul(r_ps[:, j:j + 1], lhsT, ones[:],
                                     start=first, stop=last)
                if first_chunk:
                    nc.vector.tensor_copy(o_acc[:, j, 0:512], p0[:])
                    nc.vector.tensor_copy(o_acc[:, j, 512:1024], p1[:])
                else:
                    nc.vector.tensor_add(o_acc[:, j, 0:512], o_acc[:, j, 0:512], p0[:])
                    nc.vector.tensor_add(o_acc[:, j, 512:1024], o_acc[:, j, 512:1024], p1[:])
                for _ in range(per_j):
                    item = next(next_scores, None)
                    if item is not None:
                        emit_score(item)
                if last_chunk:
                    # final fold for this q-block: finish rowsum, invert,
                    # then phase B of the PREVIOUS block (so PE never waits
                    # on this block's DVE fold)
                    nc.vector.tensor_add(r_acc[:, j:j + 1], r_acc[:, j:j + 1],
                                         r_ps[:, j:j + 1])
                    nc.vector.reciprocal(rinv[:, j:j + 1], r_acc[:, j:j + 1])
                    if j > 0:
                        emit_phase_b(j - 1)
            for item in next_scores:
                emit_score(item)
            if first_chunk:
                nc.vector.tensor_copy(r_acc[:], r_ps[:])
            elif not last_chunk:
                nc.vector.tensor_add(r_acc[:], r_acc[:], r_ps[:])

        emit_phase_b(NQS - 1)


def build_program(repeat: int = 0, scratch: int | None = 32768):
    """Build + compile the SPMD program. repeat>0 wraps the body in a
    device-side For loop (for timing) and is not used for grading."""
    from contextlib import ExitStack

    kw = {} if scratch is None else dict(dynamic_dma_scratch_size=scratch)
    nc = bacc.Bacc("TRN2", target_bir_lowering=False, debug=False, num_devices=8, **kw)

    keysT = nc.dram_tensor("keysT", [D, NKP], BF16, kind="ExternalInput")
    qvT = nc.dram_tensor("qvT", [D, QP], BF16, kind="ExternalInput")
    adj_r = nc.dram_tensor("adj_r", [P, KT, 64], I16, kind="ExternalInput")
    adj_s = nc.dram_tensor("adj_s", [P, KT, 64], I16, kind="ExternalInput")
    src_r = nc.dram_tensor("src_r", [NR, DP], BF16, kind="ExternalInput")
    src_s = nc.dram_tensor("src_s", [NS, DP], BF16, kind="ExternalInput")
    w = nc.dram_tensor("w", [P, HID // P, HID], BF16, kind="ExternalInput")
    ebias = nc.dram_tensor("ebias", [P, 1], F32, kind="ExternalInput")
    out = nc.dram_tensor("out", [QOUT, HID], F32, kind="ExternalOutput")

    tensors = (keysT, qvT, adj_r, adj_s, src_r, src_s, w, ebias, out)

    with tile.TileContext(nc) as tc, ExitStack() as ctx:
        const = ctx.enter_context(tc.tile_pool(name="const", bufs=1))
        psum = ctx.enter_context(tc.tile_pool(name="psum", bufs=5, space="PSUM"))
        psum_b = ctx.enter_context(tc.tile_pool(name="psum_b", bufs=2, space="PSUM"))
        psum_r = ctx.enter_context(tc.tile_pool(name="psum_r", bufs=1, space="PSUM"))
        pools = (const, psum, psum_b, psum_r)
        if repeat > 1:
            # device-side loop: program size stays constant as repeat grows,
            # so repeat-marginal timing isolates true body time (static
            # unrolling confounds it with per-call program-size overheads)
            with tc.For_i(0, repeat, 1):
                _emit_body(nc, tc, pools, tensors)
        else:
            _emit_body(nc, tc, pools, tensors)

    nc.compile()
    return nc


def _permute_w(w_full: np.ndarray) -> np.ndarray:
    """Reference cu columns are slot-interleaved [r0 i0 r1 i1 ...]; the kernel
    gathers [r0..r7 | i0..i7]. Permute W rows to match, then pre-tile to
    [128, 8, 1024] for the on-device layout."""
    wr = w_full.reshape(8, 2, D, HID)
    w_perm = np.concatenate(
        [wr[:, 0].reshape(8 * D, HID), wr[:, 1].reshape(8 * D, HID)], axis=0,
    )
    return np.ascontiguousarray(
        w_perm.reshape(HID // P, P, HID).transpose(1, 0, 2),
    )


def _wrap_adj(adj: np.ndarray) -> np.ndarray:
    """[6000, 8] -> [128, KT, 64] int16 in dma_gather index layout: per
    k-tile, flat index i = slot*128 + row (so gathered row i lands at
    out[i%128, i//128]), wrapped into 16 partitions (entry n at
    [n%16, n//16]) and replicated 8x to fill 128 partitions. Padded key
    rows index row 0 (their attention weight is forced to zero by ebias)."""
    a = np.zeros((NKP, 8), dtype=np.int64)
    a[:NK] = adj
    tiles = a.reshape(KT, P, 8)                                # [kt, p, c]
    idx_flat = tiles.transpose(0, 2, 1).reshape(KT, 1024)      # i = c*128+p
    idx16 = idx_flat.reshape(KT, 64, 16).transpose(0, 2, 1)    # [kt, 16, 64]
    full = np.tile(idx16, (1, 8, 1))                           # [kt, 128, 64]
    return np.ascontiguousarray(full.transpose(1, 0, 2)).astype(np.int16)


def _host_inputs(review_vecs, user_vecs, item_vecs, user_weights, item_weights,
                 user_review_adj, user_item_adj, item_review_adj, item_user_adj):
    review_vecs = np.asarray(review_vecs, dtype=np.float32)
    user_vecs = np.asarray(user_vecs, dtype=np.float32)
    item_vecs = np.asarray(item_vecs, dtype=np.float32)

    def _pad_src(v: np.ndarray) -> np.ndarray:
        out = np.zeros((v.shape[0], DP), dtype=ml_dtypes.bfloat16)
        out[:, :D] = v.astype(ml_dtypes.bfloat16)
        return out

    src_r_pad = _pad_src(review_vecs)
    sides = {}
    for side, keys, adj_r, adj_s, src_s, w_full in (
        ("user", user_vecs, user_review_adj, user_item_adj, item_vecs, user_weights),
        ("item", item_vecs, item_review_adj, item_user_adj, user_vecs, item_weights),
    ):
        keysT = np.zeros((D, NKP), dtype=ml_dtypes.bfloat16)
        keysT[:, :NK] = keys.T.astype(ml_dtypes.bfloat16)
        sides[side] = dict(
            keysT=keysT,
            adj_r=_wrap_adj(np.asarray(adj_r, dtype=np.int64)),
            adj_s=_wrap_adj(np.asarray(adj_s, dtype=np.int64)),
            src_s=_pad_src(np.asarray(src_s, dtype=np.float32)),
            w=_permute_w(np.asarray(w_full, dtype=np.float32)),
            keys=keys,
        )

    ebias = np.zeros((P, 1), dtype=np.float32)
    ebias[NK - (KT_CALC - 1) * P:] = -1e30

    in_maps = []
    for c in range(8):
        s = sides["user" if c < 4 else "item"]
        b = c % 4
        qv = s["keys"][b * QOUT:(b + 1) * QOUT]  # [1500, 64]
        qvT = np.empty((D, QP), dtype=np.float32)
        qvT[:, :QOUT] = qv.T
        qvT[:, QOUT:] = qv.T[:, :QP - QOUT]  # pad with real vectors (finite rowsums)
        in_maps.append(dict(
            keysT=s["keysT"], qvT=np.ascontiguousarray(qvT.astype(ml_dtypes.bfloat16)),
            adj_r=s["adj_r"], adj_s=s["adj_s"],
            src_r=src_r_pad, src_s=s["src_s"],
            w=s["w"].astype(ml_dtypes.bfloat16), ebias=ebias,
        ))
    return in_maps


_NC_CACHE = None


def kernel(**inputs):
    global _NC_CACHE
    if _NC_CACHE is None:
        _NC_CACHE = build_program()
    nc = _NC_CACHE
    in_maps = _host_inputs(**inputs)
    res = run_bass_kernel_spmd(nc, in_maps, core_ids=list(range(8)))
    outs = [res.results[c]["out"] for c in range(8)]
    user_output = np.concatenate(outs[0:4], axis=0)
    item_output = np.concatenate(outs[4:8], axis=0)
    return user_output, item_output


# revision 42
# speedup vs baseline: 1.8073x; 1.6769x over previous
"""Trainium2 Bass kernel for nn_AttentionAggregator.

Computation (per side, users/items symmetric):
    cu  = concat(gather(review_vecs, adj_r), gather(sec_vecs, adj_s))   # [6000, 1024]
    att = softmax(keys @ keys.T / 8) @ cu                               # [6000, 1024]
    out = relu(att @ W)                                                 # [6000, 1024]

Sharding: 8 cores run the same program (SPMD). Cores 0-3 take the user side
(1500 query rows each), cores 4-7 the item side. Keys, gather sources,
adjacency and weights are replicated; only the query slice differs.

On-device per core:
  - gather cu tile-by-tile from DRAM via ONE indirect DMA per 128-key tile
    (16 neighbor indices per key row -> 2048 descriptors per instruction;
    the r/s sources are concatenated on host so one instruction covers both)
  - scoresT[k,q] = keys @ q.T via PE in bf16 (contraction over D=64,
    zero-padded to 128)
  - E = exp(scoresT/8) on ScalarE directly PSUM->SBUF (no max-subtraction
    needed: |scores/8| <= ~5 in fp32)
  - O = E.T-weighted sum of cu, accumulated on PE in PSUM over chunks of
    k-tiles, then folded into an SBUF fp32 accumulator by DVE
  - rowsums r = E.T @ ones accumulated in a persistent PSUM bank
  - out = relu(O @ W) * (1/r), with the 1/r per-partition scale fused into
    the final ReLU PSUM->SBUF copy (valid since r > 0)

Column layout of the gathered cu is [review slots 0-7 | sec slots 0-7]
(instead of the reference's interleaved layout); the host permutes W's rows
to match, so results are identical.
"""

import os
import sys

import ml_dtypes
import numpy as np

for _p in ("/opt/trn_rl_repo", "/root/.axon_site/_ro/trn_rl_repo"):
    if os.path.isdir(_p) and _p not in sys.path:
        sys.path.append(_p)

import concourse.bass as bass  # noqa: E402
import concourse.mybir as mybir  # noqa: E402
import concourse.tile as tile  # noqa: E402
from concourse import bacc  # noqa: E402
from concourse.bass_utils import run_bass_kernel_spmd  # noqa: E402
from concourse.masks import make_identity  # noqa: E402

P = 128
D = 64
DP = 128           # gather-source row pitch (bf16 rows padded 64 -> 128 so
                   # each row is 256 B, the dma_gather stride granularity)
NK = 6000          # keys per side
NKP = 6144         # padded to 48 full k-tiles
KT = NKP // P      # 48
KT_CALC = 47       # k-tiles that carry real keys (kt 47 is all padding)
QOUT = 1500        # query rows per core (6000 / 4 cores per side)
QP = 1536          # padded to 12 full q-subtiles
NQS = QP // P      # 12
HID = 1024
NR = 30000         # review_vecs rows
NS = 6000          # secondary source rows
CHUNK_SIZES = tuple(int(x) for x in os.environ.get("K_CHUNKS", "2,4,4,4,4,4,4,4,4,4,4,4,2").split(","))
assert sum(CHUNK_SIZES) == 48
CHUNK_STARTS = tuple(int(np.cumsum((0,) + CHUNK_SIZES)[i]) for i in range(len(CHUNK_SIZES)))
F32 = mybir.dt.float32
BF16 = mybir.dt.bfloat16
I32 = mybir.dt.int32
I16 = mybir.dt.int16

AF = mybir.ActivationFunctionType


def _emit_body(nc, tc, ctx_pools, tensors):
    """Emit one full pass of the kernel body inside an open TileContext."""
    from contextlib import ExitStack

    keysT, qvT, adj_r, adj_s, src_r, src_s, w, ebias, out = tensors
    const, psum, psum_b, psum_r = ctx_pools

    # ---- persistent tiles -------------------------------------------------
    identity = const.tile([P, P], F32, tag="identity")
    make_identity(nc, identity[:])
    ones = const.tile([P, 1], BF16, tag="ones")
    nc.gpsimd.memset(ones[:], 1.0)

    # load order matters for the pipeline fill: queries + first key columns
    # first (gate the first scores), then adjacency (gates the gathers).
    # 64-partition tiles: the matmul contraction dim is just D=64, no
    # zero-padding to 128 needed.
    qvT_sb = const.tile([D, QP], BF16, tag="qvT")
    nc.sync.dma_start(qvT_sb[:, 0:512], qvT[:, 0:512])
    nc.sync.dma_start(qvT_sb[:, 512:], qvT[:, 512:])

    vecsT = const.tile([D, NKP], BF16, tag="vecsT")
    nc.sync.dma_start(vecsT[:, :NKP // 8], keysT[:, :NKP // 8])

    adj_r_sb = const.tile([P, KT, 64], I16, tag="adjr")
    nc.sync.dma_start(adj_r_sb[:], adj_r[:, :, :])
    adj_s_sb = const.tile([P, KT, 64], I16, tag="adjs")
    nc.sync.dma_start(adj_s_sb[:], adj_s[:, :, :])

    nc.sync.dma_start(vecsT[:, NKP // 8:NKP // 2], keysT[:, NKP // 8:NKP // 2])
    nc.sync.dma_start(vecsT[:, NKP // 2:], keysT[:, NKP // 2:])

    ebias_sb = const.tile([P, 1], F32, tag="ebias")
    nc.sync.dma_start(ebias_sb[:], ebias[:, :])

    o_acc = const.tile([P, NQS, HID], F32, tag="oacc")
    r_acc = const.tile([P, NQS], F32, tag="racc")
    rinv = const.tile([P, NQS], F32, tag="rinv")

    w_sb = const.tile([P, HID // P, HID], BF16, tag="w")
    nc.sync.dma_start(w_sb[:], w[:, :, :])

    chunks = [list(range(st, min(st + cs, KT_CALC)))
              for st, cs in zip(CHUNK_STARTS, CHUNK_SIZES)]
    chunks = [c for c in chunks if c]

    with ExitStack() as ctx:
        e_pool = ctx.enter_context(tc.tile_pool(name="e_pool", bufs=12))
        g_pool = ctx.enter_context(tc.tile_pool(name="g_pool", bufs=12))
        ot_pool = ctx.enter_context(tc.tile_pool(name="ot_pool", bufs=4))
        ob_pool = ctx.enter_context(tc.tile_pool(name="ob_pool", bufs=3))

        e_tiles = {}
        g_tiles = {}
        ablate = os.environ.get("K_ABLATE", "")
        if ablate == "nogather":
            gr0 = const.tile([P, 8, DP], BF16, tag="gr0")
            gs0 = const.tile([P, 8, DP], BF16, tag="gs0")
            nc.any.memzero(gr0[:])
            nc.any.memzero(gs0[:])

        def emit_gather(chunk):
            if ablate == "nogather":
                for kt in chunk:
                    g_tiles[kt] = (gr0, gs0)
                return
            # two dma_gathers per k-tile (8 review + 8 secondary neighbor
            # rows per key row): 1024 indices each, one 256 B padded source
            # row per index. Result tiles are [128, 8 slots, 128] with the
            # payload in cols 0:64 of each slot (the numerator matmul reads
            # the strided [.., 0:64] view directly).
            for kt in chunk:
                gr = g_pool.tile([P, 8, DP], BF16, tag="gr")
                nc.gpsimd.dma_gather(
                    out_ap=gr[:],
                    in_ap=src_r[:],
                    idxs_ap=adj_r_sb[:, kt, :],
                    num_idxs=1024,
                    num_idxs_reg=1024,
                    elem_size=DP,
                    queue_num=(2 * kt) % 4,
                )
                gs = g_pool.tile([P, 8, DP], BF16, tag="gs")
                nc.gpsimd.dma_gather(
                    out_ap=gs[:],
                    in_ap=src_s[:],
                    idxs_ap=adj_s_sb[:, kt, :],
                    num_idxs=1024,
                    num_idxs_reg=1024,
                    elem_size=DP,
                    queue_num=(2 * kt + 1) % 4,
                )
                g_tiles[kt] = (gr, gs)

        def score_items(chunk):
            """One (matmul+exp) emission per item; consumed a few per
            numerator iteration so the exps overlap the previous chunk's
            numerator instead of serializing before this chunk's."""
            for kt in chunk:
                e = e_pool.tile([P, QP], BF16, tag="e")
                e_tiles[kt] = e
                for i in range(QP // 512):
                    yield kt, e, i

        def emit_score(item):
            kt, e, i = item
            lhsT = vecsT[:, kt * P:(kt + 1) * P]  # [64, 128]: K=64 contraction
            s_ps = psum.tile([P, 512], F32, tag="ps")
            nc.tensor.matmul(
                s_ps[:], lhsT, qvT_sb[:, i * 512:(i + 1) * 512],
                start=True, stop=True,
            )
            # padded key rows (6000..6015) get bias -1e30 so exp()
            # forces their attention weight to exactly zero
            bias = ebias_sb[:, 0:1] if kt == KT_CALC - 1 else 0.0
            nc.scalar.activation(
                e[:, i * 512:(i + 1) * 512], s_ps[:], AF.Exp,
                bias=bias, scale=0.125,
            )

        def emit_phase_b(j):
            """Transpose o_acc[:, j], project through W, relu*1/r, store."""
            ots = []
            for half in range(2):
                tp = psum_b.tile([P, 512], F32, tag="pb")
                for t in range(4):
                    nc.tensor.transpose(
                        tp[:, t * P:(t + 1) * P],
                        o_acc[:, j, (half * 4 + t) * P:(half * 4 + t + 1) * P],
                        identity[:],
                    )
                ot = ot_pool.tile([P, 512], BF16, tag="ot")
                nc.vector.tensor_copy(ot[:], tp[:])
                ots.append(ot)
            for h in range(HID // 512):
                pf = psum_b.tile([P, 512], F32, tag="pb")
                for t in range(HID // P):
                    nc.tensor.matmul(
                        pf[:], ots[t // 4][:, (t % 4) * P:(t % 4 + 1) * P],
                        w_sb[:, t, h * 512:(h + 1) * 512],
                        start=(t == 0), stop=(t == HID // P - 1),
                    )
                ob = ob_pool.tile([P, 512], F32, tag="ob")
                nc.scalar.activation(ob[:], pf[:], AF.Relu, scale=rinv[:, j:j + 1])
                rows = min(P, QOUT - j * P)
                if rows > 0:
                    nc.sync.dma_start(
                        out[j * P:j * P + rows, h * 512:(h + 1) * 512], ob[:rows, :],
                    )

        emit_gather(chunks[0])
        if ablate != "gatheronly":
            # scores two chunks ahead: chunk 0 and 1 up front, then chunk
            # i+2 interleaved into chunk i's numerator -- PE always has a
            # full chunk of scored keys buffered against gather jitter
            for item in score_items(chunks[0]):
                emit_score(item)
            for item in score_items(chunks[1]):
                emit_score(item)

        for ci, chunk in enumerate(chunks):
            first_chunk = ci == 0
            last_chunk = ci == len(chunks) - 1
            if not last_chunk:
                emit_gather(chunks[ci + 1])
            if ci + 2 < len(chunks):
                next_scores = score_items(chunks[ci + 2])
                per_j = -(-3 * len(chunks[ci + 2]) // NQS)
            else:
                next_scores = iter(())
                per_j = 0

            if ablate == "gatheronly":
                continue
            r_ps = psum_r.tile([P, NQS], F32, tag="rps")
            for j in range(NQS):
                p0 = psum.tile([P, 512], F32, tag="ps")
                p1 = psum.tile([P, 512], F32, tag="ps")
                for i, kt in enumerate(chunk):
                    lhsT = e_tiles[kt][:, j * P:(j + 1) * P]
                    first = i == 0
                    last = i == len(chunk) - 1
                    gr, gs = g_tiles[kt]
                    nc.tensor.matmul(p0[:], lhsT, gr[:, :, 0:D],
                                     start=first, stop=last)
                    nc.tensor.matmul(p1[:], lhsT, gs[:, :, 0:D],
                                     start=first, stop=last)
                    nc.tensor.matmul(r_ps[:, j:j + 1], lhsT, ones[:],
                                     start=first, stop=last)
                if first_chunk:
                    nc.vector.tensor_copy(o_acc[:, j, 0:512], p0[:])
                    nc.vector.tensor_copy(o_acc[:, j, 512:1024], p1[:])
                else:
                    nc.vector.tensor_add(o_acc[:, j, 0:512], o_acc[:, j, 0:512], p0[:])
                    nc.vector.tensor_add(o_acc[:, j, 512:1024], o_acc[:, j, 512:1024], p1[:])
                for _ in range(per_j):
                    item = next(next_scores, None)
                    if item is not None:
                        emit_score(item)
                if last_chunk:
                    # final fold for this q-block: finish rowsum, invert,
                    # then phase B of the PREVIOUS block (so PE never waits
                    # on this block's DVE fold)
                    nc.vector.tensor_add(r_acc[:, j:j + 1], r_acc[:, j:j + 1],
                                         r_ps[:, j:j + 1])
                    nc.vector.reciprocal(rinv[:, j:j + 1], r_acc[:, j:j + 1])
                    if j > 0:
                        emit_phase_b(j - 1)
            for item in next_scores:
                emit_score(item)
            if first_chunk:
                nc.vector.tensor_copy(r_acc[:], r_ps[:])
            elif not last_chunk:
                nc.vector.tensor_add(r_acc[:], r_acc[:], r_ps[:])

        if ablate != "gatheronly":
            emit_phase_b(NQS - 1)


def build_program(repeat: int = 0, scratch: int | None = 32768):
    """Build + compile the SPMD program. repeat>0 wraps the body in a
    device-side For loop (for timing) and is not used for grading."""
    from contextlib import ExitStack

    kw = {} if scratch is None else dict(dynamic_dma_scratch_size=scratch)
    nc = bacc.Bacc("TRN2", target_bir_lowering=False, debug=False, num_devices=8,
                   num_swdge_queues=4, **kw)

    keysT = nc.dram_tensor("keysT", [D, NKP], BF16, kind="ExternalInput")
    qvT = nc.dram_tensor("qvT", [D, QP], BF16, kind="ExternalInput")
    adj_r = nc.dram_tensor("adj_r", [P, KT, 64], I16, kind="ExternalInput")
    adj_s = nc.dram_tensor("adj_s", [P, KT, 64], I16, kind="ExternalInput")
    src_r = nc.dram_tensor("src_r", [NR, DP], BF16, kind="ExternalInput")
    src_s = nc.dram_tensor("src_s", [NS, DP], BF16, kind="ExternalInput")
    w = nc.dram_tensor("w", [P, HID // P, HID], BF16, kind="ExternalInput")
    ebias = nc.dram_tensor("ebias", [P, 1], F32, kind="ExternalInput")
    out = nc.dram_tensor("out", [QOUT, HID], F32, kind="ExternalOutput")

    tensors = (keysT, qvT, adj_r, adj_s, src_r, src_s, w, ebias, out)

    with tile.TileContext(nc) as tc, ExitStack() as ctx:
        const = ctx.enter_context(tc.tile_pool(name="const", bufs=1))
        psum = ctx.enter_context(tc.tile_pool(name="psum", bufs=4, space="PSUM"))
        psum_b = ctx.enter_context(tc.tile_pool(name="psum_b", bufs=2, space="PSUM"))
        psum_r = ctx.enter_context(tc.tile_pool(name="psum_r", bufs=2, space="PSUM"))
        pools = (const, psum, psum_b, psum_r)
        if repeat > 1:
            # device-side loop: program size stays constant as repeat grows,
            # so repeat-marginal timing isolates true body time (static
            # unrolling confounds it with per-call program-size overheads)
            with tc.For_i(0, repeat, 1):
                _emit_body(nc, tc, pools, tensors)
        else:
            _emit_body(nc, tc, pools, tensors)

    nc.compile()
    return nc


def _permute_w(w_full: np.ndarray) -> np.ndarray:
    """Reference cu columns are slot-interleaved [r0 i0 r1 i1 ...]; the kernel
    gathers [r0..r7 | i0..i7]. Permute W rows to match, then pre-tile to
    [128, 8, 1024] for the on-device layout."""
    wr = w_full.reshape(8, 2, D, HID)
    w_perm = np.concatenate(
        [wr[:, 0].reshape(8 * D, HID), wr[:, 1].reshape(8 * D, HID)], axis=0,
    )
    return np.ascontiguousarray(
        w_perm.reshape(HID // P, P, HID).transpose(1, 0, 2),
    )


def _wrap_adj(adj: np.ndarray) -> np.ndarray:
    """[6000, 8] -> [128, KT, 64] int16 in dma_gather index layout: per
    k-tile, flat index i = slot*128 + row (so gathered row i lands at
    out[i%128, i//128]), wrapped into 16 partitions (entry n at
    [n%16, n//16]) and replicated 8x to fill 128 partitions. Padded key
    rows index row 0 (their attention weight is forced to zero by ebias)."""
    a = np.zeros((NKP, 8), dtype=np.int64)
    a[:NK] = adj
    tiles = a.reshape(KT, P, 8)                                # [kt, p, c]
    idx_flat = tiles.transpose(0, 2, 1).reshape(KT, 1024)      # i = c*128+p
    idx16 = idx_flat.reshape(KT, 64, 16).transpose(0, 2, 1)    # [kt, 16, 64]
    full = np.tile(idx16, (1, 8, 1))                           # [kt, 128, 64]
    return np.ascontiguousarray(full.transpose(1, 0, 2)).astype(np.int16)


def _host_inputs(review_vecs, user_vecs, item_vecs, user_weights, item_weights,
                 user_review_adj, user_item_adj, item_review_adj, item_user_adj):
    review_vecs = np.asarray(review_vecs, dtype=np.float32)
    user_vecs = np.asarray(user_vecs, dtype=np.float32)
    item_vecs = np.asarray(item_vecs, dtype=np.float32)

    def _pad_src(v: np.ndarray) -> np.ndarray:
        out = np.zeros((v.shape[0], DP), dtype=ml_dtypes.bfloat16)
        out[:, :D] = v.astype(ml_dtypes.bfloat16)
        return out

    src_r_pad = _pad_src(review_vecs)
    sides = {}
    for side, keys, adj_r, adj_s, src_s, w_full in (
        ("user", user_vecs, user_review_adj, user_item_adj, item_vecs, user_weights),
        ("item", item_vecs, item_review_adj, item_user_adj, user_vecs, item_weights),
    ):
        keysT = np.zeros((D, NKP), dtype=ml_dtypes.bfloat16)
        keysT[:, :NK] = keys.T.astype(ml_dtypes.bfloat16)
        sides[side] = dict(
            keysT=keysT,
            adj_r=_wrap_adj(np.asarray(adj_r, dtype=np.int64)),
            adj_s=_wrap_adj(np.asarray(adj_s, dtype=np.int64)),
            src_s=_pad_src(np.asarray(src_s, dtype=np.float32)),
            w=_permute_w(np.asarray(w_full, dtype=np.float32)),
            keys=keys,
        )

    ebias = np.zeros((P, 1), dtype=np.float32)
    ebias[NK - (KT_CALC - 1) * P:] = -1e30

    in_maps = []
    for c in range(8):
        s = sides["user" if c < 4 else "item"]
        b = c % 4
        qv = s["keys"][b * QOUT:(b + 1) * QOUT]  # [1500, 64]
        qvT = np.empty((D, QP), dtype=np.float32)
        qvT[:, :QOUT] = qv.T
        qvT[:, QOUT:] = qv.T[:, :QP - QOUT]  # pad with real vectors (finite rowsums)
        in_maps.append(dict(
            keysT=s["keysT"], qvT=np.ascontiguousarray(qvT.astype(ml_dtypes.bfloat16)),
            adj_r=s["adj_r"], adj_s=s["adj_s"],
            src_r=src_r_pad, src_s=s["src_s"],
            w=s["w"].astype(ml_dtypes.bfloat16), ebias=ebias,
        ))
    return in_maps


_NC_CACHE = None


def kernel(**inputs):
    global _NC_CACHE
    if _NC_CACHE is None:
        _NC_CACHE = build_program()
    nc = _NC_CACHE
    in_maps = _host_inputs(**inputs)
    res = run_bass_kernel_spmd(nc, in_maps, core_ids=list(range(8)))
    outs = [res.results[c]["out"] for c in range(8)]
    user_output = np.concatenate(outs[0:4], axis=0)
    item_output = np.concatenate(outs[4:8], axis=0)
    return user_output, item_output


# revision 43
# speedup vs baseline: 2.0675x; 1.1440x over previous
"""Trainium2 Bass kernel for nn_AttentionAggregator.

Computation (per side, users/items symmetric):
    cu  = concat(gather(review_vecs, adj_r), gather(sec_vecs, adj_s))   # [6000, 1024]
    att = softmax(keys @ keys.T / 8) @ cu                               # [6000, 1024]
    out = relu(att @ W)                                                 # [6000, 1024]

Sharding: 8 cores run the same program (SPMD). Cores 0-3 take the user side
(1500 query rows each), cores 4-7 the item side. Keys, gather sources,
adjacency and weights are replicated; only the query slice differs.

On-device per core:
  - gather cu tile-by-tile from DRAM via dma_gather (InstDMAGatherAnt custom
    ucode): 2 instructions per 128-key tile (review/secondary), 1024 int16
    indices each, one padded 256 B source row per index. Gathers are spread
    round-robin over 4 SWDGE queues (single-queue descriptor generation at
    ~8-12 ns/idx was the original kernel's bottleneck; 4 queues + 1024-idx
    batching brings the full 98k-index gather under ~90 us)
  - scoresT[k,q] = keys @ q.T via PE in bf16 (64-partition contraction)
  - E = exp(scoresT/8) on ScalarE directly PSUM->SBUF (no max-subtraction
    needed: |scores/8| <= ~5 in fp32); scores run two chunks ahead of the
    numerator so the exps overlap PE's numerator matmuls
  - O = E.T-weighted sum of cu, accumulated on PE in PSUM over chunks of
    k-tiles, then folded into an SBUF fp32 accumulator by DVE. The matmul
    reads the gathered tiles' [128, 8, 0:64] strided view (cols 64:128 of
    each 256 B row are pad)
  - rowsums r = E.T @ ones accumulated in a persistent PSUM bank
  - out = relu(O @ W) * (1/r), with the 1/r per-partition scale fused into
    the final ReLU PSUM->SBUF copy (valid since r > 0); phase B for q-block
    j-1 is interleaved into the last chunk's numerator iteration j

Column layout of the gathered cu is [review slots 0-7 | sec slots 0-7]
(instead of the reference's interleaved layout); the host permutes W's rows
to match, so results are identical.
"""

import os
import sys

import ml_dtypes
import numpy as np

for _p in ("/opt/trn_rl_repo", "/root/.axon_site/_ro/trn_rl_repo"):
    if os.path.isdir(_p) and _p not in sys.path:
        sys.path.append(_p)

import concourse.bass as bass  # noqa: E402
import concourse.mybir as mybir  # noqa: E402
import concourse.tile as tile  # noqa: E402
from concourse import bacc  # noqa: E402
from concourse.bass_utils import run_bass_kernel_spmd  # noqa: E402
from concourse.masks import make_identity  # noqa: E402

P = 128
D = 64
DP = 128           # gather-source row pitch (bf16 rows padded 64 -> 128 so
                   # each row is 256 B, the dma_gather stride granularity)
NK = 6000          # keys per side
NKP = 6144         # padded to 48 full k-tiles
KT = NKP // P      # 48
KT_CALC = 47       # k-tiles that carry real keys (kt 47 is all padding)
QOUT = 1500        # query rows per core (6000 / 4 cores per side)
QP = 1536          # padded to 12 full q-subtiles
NQS = QP // P      # 12
HID = 1024
NR = 30000         # review_vecs rows
NS = 6000          # secondary source rows
CHUNK_SIZES = tuple(int(x) for x in os.environ.get("K_CHUNKS", "2,4,4,4,4,4,4,4,4,4,4,4,2").split(","))
assert sum(CHUNK_SIZES) == 48
CHUNK_STARTS = tuple(int(np.cumsum((0,) + CHUNK_SIZES)[i]) for i in range(len(CHUNK_SIZES)))
F32 = mybir.dt.float32
BF16 = mybir.dt.bfloat16
I32 = mybir.dt.int32
I16 = mybir.dt.int16

AF = mybir.ActivationFunctionType


def _emit_body(nc, tc, ctx_pools, tensors):
    """Emit one full pass of the kernel body inside an open TileContext."""
    from contextlib import ExitStack

    keysT, qvT, adj_r, adj_s, src_r, src_s, w, ebias, out = tensors
    const, psum, psum_b, psum_r = ctx_pools

    # ---- persistent tiles -------------------------------------------------
    identity = const.tile([P, P], F32, tag="identity")
    make_identity(nc, identity[:])
    ones = const.tile([P, 1], BF16, tag="ones")
    nc.gpsimd.memset(ones[:], 1.0)

    # load order matters for the pipeline fill: queries + first key columns
    # first (gate the first scores), then adjacency (gates the gathers).
    # 64-partition tiles: the matmul contraction dim is just D=64, no
    # zero-padding to 128 needed.
    qvT_sb = const.tile([D, QP], BF16, tag="qvT")
    nc.sync.dma_start(qvT_sb[:, 0:512], qvT[:, 0:512])
    nc.sync.dma_start(qvT_sb[:, 512:], qvT[:, 512:])

    vecsT = const.tile([D, NKP], BF16, tag="vecsT")
    nc.sync.dma_start(vecsT[:, :NKP // 8], keysT[:, :NKP // 8])

    adj_r_sb = const.tile([P, KT, 64], I16, tag="adjr")
    nc.sync.dma_start(adj_r_sb[:], adj_r[:, :, :])
    adj_s_sb = const.tile([P, KT, 64], I16, tag="adjs")
    nc.sync.dma_start(adj_s_sb[:], adj_s[:, :, :])

    nc.sync.dma_start(vecsT[:, NKP // 8:NKP // 2], keysT[:, NKP // 8:NKP // 2])
    nc.sync.dma_start(vecsT[:, NKP // 2:], keysT[:, NKP // 2:])

    ebias_sb = const.tile([P, 1], F32, tag="ebias")
    nc.sync.dma_start(ebias_sb[:], ebias[:, :])

    o_acc = const.tile([P, NQS, HID], F32, tag="oacc")
    r_acc = const.tile([P, NQS], F32, tag="racc")
    rinv = const.tile([P, NQS], F32, tag="rinv")

    w_sb = const.tile([P, HID // P, HID], BF16, tag="w")
    nc.sync.dma_start(w_sb[:], w[:, :, :])

    chunks = [list(range(st, min(st + cs, KT_CALC)))
              for st, cs in zip(CHUNK_STARTS, CHUNK_SIZES)]
    chunks = [c for c in chunks if c]

    with ExitStack() as ctx:
        e_pool = ctx.enter_context(tc.tile_pool(name="e_pool", bufs=12))
        g_pool = ctx.enter_context(tc.tile_pool(name="g_pool", bufs=12))
        ot_pool = ctx.enter_context(tc.tile_pool(name="ot_pool", bufs=4))
        ob_pool = ctx.enter_context(tc.tile_pool(name="ob_pool", bufs=3))

        e_tiles = {}
        g_tiles = {}
        ablate = os.environ.get("K_ABLATE", "")
        if ablate == "nogather":
            gr0 = const.tile([P, 8, DP], BF16, tag="gr0")
            gs0 = const.tile([P, 8, DP], BF16, tag="gs0")
            nc.any.memzero(gr0[:])
            nc.any.memzero(gs0[:])

        def emit_gather(chunk):
            if ablate == "nogather":
                for kt in chunk:
                    g_tiles[kt] = (gr0, gs0)
                return
            # two dma_gathers per k-tile (8 review + 8 secondary neighbor
            # rows per key row): 1024 indices each, one 256 B padded source
            # row per index. Result tiles are [128, 8 slots, 128] with the
            # payload in cols 0:64 of each slot (the numerator matmul reads
            # the strided [.., 0:64] view directly).
            for kt in chunk:
                gr = g_pool.tile([P, 8, DP], BF16, tag="gr")
                nc.gpsimd.dma_gather(
                    out_ap=gr[:],
                    in_ap=src_r[:],
                    idxs_ap=adj_r_sb[:, kt, :],
                    num_idxs=1024,
                    num_idxs_reg=1024,
                    elem_size=DP,
                    queue_num=(2 * kt) % 4,
                )
                gs = g_pool.tile([P, 8, DP], BF16, tag="gs")
                nc.gpsimd.dma_gather(
                    out_ap=gs[:],
                    in_ap=src_s[:],
                    idxs_ap=adj_s_sb[:, kt, :],
                    num_idxs=1024,
                    num_idxs_reg=1024,
                    elem_size=DP,
                    queue_num=(2 * kt + 1) % 4,
                )
                g_tiles[kt] = (gr, gs)

        def score_items(chunk):
            """One (matmul+exp) emission per item; consumed a few per
            numerator iteration so the exps overlap the previous chunk's
            numerator instead of serializing before this chunk's."""
            for kt in chunk:
                e = e_pool.tile([P, QP], BF16, tag="e")
                e_tiles[kt] = e
                for i in range(QP // 512):
                    yield kt, e, i

        def emit_score(item):
            kt, e, i = item
            lhsT = vecsT[:, kt * P:(kt + 1) * P]  # [64, 128]: K=64 contraction
            s_ps = psum.tile([P, 512], F32, tag="ps")
            nc.tensor.matmul(
                s_ps[:], lhsT, qvT_sb[:, i * 512:(i + 1) * 512],
                start=True, stop=True,
            )
            # padded key rows (6000..6015) get bias -1e30 so exp()
            # forces their attention weight to exactly zero
            bias = ebias_sb[:, 0:1] if kt == KT_CALC - 1 else 0.0
            nc.scalar.activation(
                e[:, i * 512:(i + 1) * 512], s_ps[:], AF.Exp,
                bias=bias, scale=0.125,
            )

        def emit_phase_b(j):
            """Transpose o_acc[:, j], project through W, relu*1/r, store."""
            ots = []
            for half in range(2):
                tp = psum_b.tile([P, 512], F32, tag="pb")
                for t in range(4):
                    nc.tensor.transpose(
                        tp[:, t * P:(t + 1) * P],
                        o_acc[:, j, (half * 4 + t) * P:(half * 4 + t + 1) * P],
                        identity[:],
                    )
                ot = ot_pool.tile([P, 512], BF16, tag="ot")
                nc.vector.tensor_copy(ot[:], tp[:])
                ots.append(ot)
            for h in range(HID // 512):
                pf = psum_b.tile([P, 512], F32, tag="pb")
                for t in range(HID // P):
                    nc.tensor.matmul(
                        pf[:], ots[t // 4][:, (t % 4) * P:(t % 4 + 1) * P],
                        w_sb[:, t, h * 512:(h + 1) * 512],
                        start=(t == 0), stop=(t == HID // P - 1),
                    )
                ob = ob_pool.tile([P, 512], F32, tag="ob")
                nc.scalar.activation(ob[:], pf[:], AF.Relu, scale=rinv[:, j:j + 1])
                rows = min(P, QOUT - j * P)
                if rows > 0:
                    nc.sync.dma_start(
                        out[j * P:j * P + rows, h * 512:(h + 1) * 512], ob[:rows, :],
                    )

        emit_gather(chunks[0])
        if ablate != "gatheronly":
            # scores two chunks ahead: chunk 0 and 1 up front, then chunk
            # i+2 interleaved into chunk i's numerator -- PE always has a
            # full chunk of scored keys buffered against gather jitter
            for item in score_items(chunks[0]):
                emit_score(item)
            for item in score_items(chunks[1]):
                emit_score(item)

        for ci, chunk in enumerate(chunks):
            first_chunk = ci == 0
            last_chunk = ci == len(chunks) - 1
            if not last_chunk:
                emit_gather(chunks[ci + 1])
            if ci + 2 < len(chunks):
                next_scores = score_items(chunks[ci + 2])
                per_j = -(-3 * len(chunks[ci + 2]) // NQS)
            else:
                next_scores = iter(())
                per_j = 0

            if ablate == "gatheronly":
                continue
            r_ps = psum_r.tile([P, NQS], F32, tag="rps")
            for j in range(NQS):
                p0 = psum.tile([P, 512], F32, tag="ps")
                p1 = psum.tile([P, 512], F32, tag="ps")
                for i, kt in enumerate(chunk):
                    lhsT = e_tiles[kt][:, j * P:(j + 1) * P]
                    first = i == 0
                    last = i == len(chunk) - 1
                    gr, gs = g_tiles[kt]
                    nc.tensor.matmul(p0[:], lhsT, gr[:, :, 0:D],
                                     start=first, stop=last)
                    nc.tensor.matmul(p1[:], lhsT, gs[:, :, 0:D],
                                     start=first, stop=last)
                    nc.tensor.matmul(r_ps[:, j:j + 1], lhsT, ones[:],
                                     start=first, stop=last)
                if first_chunk:
                    nc.vector.tensor_copy(o_acc[:, j, 0:512], p0[:])
                    nc.vector.tensor_copy(o_acc[:, j, 512:1024], p1[:])
                else:
                    nc.vector.tensor_add(o_acc[:, j, 0:512], o_acc[:, j, 0:512], p0[:])
                    nc.vector.tensor_add(o_acc[:, j, 512:1024], o_acc[:, j, 512:1024], p1[:])
                for _ in range(per_j):
                    item = next(next_scores, None)
                    if item is not None:
                        emit_score(item)
                if last_chunk:
                    # final fold for this q-block: finish rowsum, invert,
                    # then phase B of the PREVIOUS block (so PE never waits
                    # on this block's DVE fold)
                    nc.vector.tensor_add(r_acc[:, j:j + 1], r_acc[:, j:j + 1],
                                         r_ps[:, j:j + 1])
                    nc.vector.reciprocal(rinv[:, j:j + 1], r_acc[:, j:j + 1])
                    if j > 0:
                        emit_phase_b(j - 1)
            for item in next_scores:
                emit_score(item)
            if first_chunk:
                nc.vector.tensor_copy(r_acc[:], r_ps[:])
            elif not last_chunk:
                nc.vector.tensor_add(r_acc[:], r_acc[:], r_ps[:])

        if ablate != "gatheronly":
            emit_phase_b(NQS - 1)


def build_program(repeat: int = 0, scratch: int | None = 32768):
    """Build + compile the SPMD program. repeat>0 wraps the body in a
    device-side For loop (for timing) and is not used for grading."""
    from contextlib import ExitStack

    kw = {} if scratch is None else dict(dynamic_dma_scratch_size=scratch)
    nc = bacc.Bacc("TRN2", target_bir_lowering=False, debug=False, num_devices=8,
                   num_swdge_queues=4, **kw)

    keysT = nc.dram_tensor("keysT", [D, NKP], BF16, kind="ExternalInput")
    qvT = nc.dram_tensor("qvT", [D, QP], BF16, kind="ExternalInput")
    adj_r = nc.dram_tensor("adj_r", [P, KT, 64], I16, kind="ExternalInput")
    adj_s = nc.dram_tensor("adj_s", [P, KT, 64], I16, kind="ExternalInput")
    src_r = nc.dram_tensor("src_r", [NR, DP], BF16, kind="ExternalInput")
    src_s = nc.dram_tensor("src_s", [NS, DP], BF16, kind="ExternalInput")
    w = nc.dram_tensor("w", [P, HID // P, HID], BF16, kind="ExternalInput")
    ebias = nc.dram_tensor("ebias", [P, 1], F32, kind="ExternalInput")
    out = nc.dram_tensor("out", [QOUT, HID], F32, kind="ExternalOutput")

    tensors = (keysT, qvT, adj_r, adj_s, src_r, src_s, w, ebias, out)

    with tile.TileContext(nc) as tc, ExitStack() as ctx:
        const = ctx.enter_context(tc.tile_pool(name="const", bufs=1))
        psum = ctx.enter_context(tc.tile_pool(name="psum", bufs=4, space="PSUM"))
        psum_b = ctx.enter_context(tc.tile_pool(name="psum_b", bufs=2, space="PSUM"))
        psum_r = ctx.enter_context(tc.tile_pool(name="psum_r", bufs=2, space="PSUM"))
        pools = (const, psum, psum_b, psum_r)
        if repeat > 1:
            # device-side loop: program size stays constant as repeat grows,
            # so repeat-marginal timing isolates true body time (static
            # unrolling confounds it with per-call program-size overheads)
            with tc.For_i(0, repeat, 1):
                _emit_body(nc, tc, pools, tensors)
        else:
            _emit_body(nc, tc, pools, tensors)

    nc.compile()
    return nc


def _permute_w(w_full: np.ndarray) -> np.ndarray:
    """Reference cu columns are slot-interleaved [r0 i0 r1 i1 ...]; the kernel
    gathers [r0..r7 | i0..i7]. Permute W rows to match, then pre-tile to
    [128, 8, 1024] for the on-device layout."""
    wr = w_full.reshape(8, 2, D, HID)
    w_perm = np.concatenate(
        [wr[:, 0].reshape(8 * D, HID), wr[:, 1].reshape(8 * D, HID)], axis=0,
    )
    return np.ascontiguousarray(
        w_perm.reshape(HID // P, P, HID).transpose(1, 0, 2),
    )


def _wrap_adj(adj: np.ndarray) -> np.ndarray:
    """[6000, 8] -> [128, KT, 64] int16 in dma_gather index layout: per
    k-tile, flat index i = slot*128 + row (so gathered row i lands at
    out[i%128, i//128]), wrapped into 16 partitions (entry n at
    [n%16, n//16]) and replicated 8x to fill 128 partitions. Padded key
    rows index row 0 (their attention weight is forced to zero by ebias)."""
    a = np.zeros((NKP, 8), dtype=np.int64)
    a[:NK] = adj
    tiles = a.reshape(KT, P, 8)                                # [kt, p, c]
    idx_flat = tiles.transpose(0, 2, 1).reshape(KT, 1024)      # i = c*128+p
    idx16 = idx_flat.reshape(KT, 64, 16).transpose(0, 2, 1)    # [kt, 16, 64]
    full = np.tile(idx16, (1, 8, 1))                           # [kt, 128, 64]
    return np.ascontiguousarray(full.transpose(1, 0, 2)).astype(np.int16)


def _host_inputs(review_vecs, user_vecs, item_vecs, user_weights, item_weights,
                 user_review_adj, user_item_adj, item_review_adj, item_user_adj):
    review_vecs = np.asarray(review_vecs, dtype=np.float32)
    user_vecs = np.asarray(user_vecs, dtype=np.float32)
    item_vecs = np.asarray(item_vecs, dtype=np.float32)

    def _pad_src(v: np.ndarray) -> np.ndarray:
        out = np.zeros((v.shape[0], DP), dtype=ml_dtypes.bfloat16)
        out[:, :D] = v.astype(ml_dtypes.bfloat16)
        return out

    src_r_pad = _pad_src(review_vecs)
    sides = {}
    for side, keys, adj_r, adj_s, src_s, w_full in (
        ("user", user_vecs, user_review_adj, user_item_adj, item_vecs, user_weights),
        ("item", item_vecs, item_review_adj, item_user_adj, user_vecs, item_weights),
    ):
        keysT = np.zeros((D, NKP), dtype=ml_dtypes.bfloat16)
        keysT[:, :NK] = keys.T.astype(ml_dtypes.bfloat16)
        sides[side] = dict(
            keysT=keysT,
            adj_r=_wrap_adj(np.asarray(adj_r, dtype=np.int64)),
            adj_s=_wrap_adj(np.asarray(adj_s, dtype=np.int64)),
            src_s=_pad_src(np.asarray(src_s, dtype=np.float32)),
            w=_permute_w(np.asarray(w_full, dtype=np.float32)),
            keys=keys,
        )

    ebias = np.zeros((P, 1), dtype=np.float32)
    ebias[NK - (KT_CALC - 1) * P:] = -1e30

    in_maps = []
    for c in range(8):
        s = sides["user" if c < 4 else "item"]
        b = c % 4
        qv = s["keys"][b * QOUT:(b + 1) * QOUT]  # [1500, 64]
        qvT = np.empty((D, QP), dtype=np.float32)
        qvT[:, :QOUT] = qv.T
        qvT[:, QOUT:] = qv.T[:, :QP - QOUT]  # pad with real vectors (finite rowsums)
        in_maps.append(dict(
            keysT=s["keysT"], qvT=np.ascontiguousarray(qvT.astype(ml_dtypes.bfloat16)),
            adj_r=s["adj_r"], adj_s=s["adj_s"],
            src_r=src_r_pad, src_s=s["src_s"],
            w=s["w"].astype(ml_dtypes.bfloat16), ebias=ebias,
        ))
    return in_maps


_NC_CACHE = None


def kernel(**inputs):
    global _NC_CACHE
    if _NC_CACHE is None:
        _NC_CACHE = build_program()
    nc = _NC_CACHE
    in_maps = _host_inputs(**inputs)
    res = run_bass_kernel_spmd(nc, in_maps, core_ids=list(range(8)))
    outs = [res.results[c]["out"] for c in range(8)]
    user_output = np.concatenate(outs[0:4], axis=0)
    item_output = np.concatenate(outs[4:8], axis=0)
    return user_output, item_output


# revision 55
# speedup vs baseline: 2.6139x; 1.2643x over previous
"""Trainium2 Bass kernel for nn_AttentionAggregator.

Computation (per side, users/items symmetric):
    cu  = concat(gather(review_vecs, adj_r), gather(sec_vecs, adj_s))   # [6000, 1024]
    att = softmax(keys @ keys.T / 8) @ cu                               # [6000, 1024]
    out = relu(att @ W)                                                 # [6000, 1024]

Sharding: 8 cores run the same program (SPMD). Cores 0-3 take the user side
(1500 query rows each), cores 4-7 the item side. Keys, gather sources,
adjacency and weights are replicated; only the query slice differs.

On-device per core:
  - gather cu tile-by-tile from DRAM via dma_gather (InstDMAGatherAnt custom
    ucode): 2 instructions per 128-key tile (review/secondary), 1024 int16
    indices each, one padded 256 B source row per index. Gathers are spread
    round-robin over 4 SWDGE queues (single-queue descriptor generation at
    ~8-12 ns/idx was the original kernel's bottleneck; 4 queues + 1024-idx
    batching brings the full 98k-index gather under ~90 us)
  - scoresT[k,q] = keys @ q.T via PE in bf16 (64-partition contraction)
  - E = exp(scoresT/8) on ScalarE directly PSUM->SBUF (no max-subtraction
    needed: |scores/8| <= ~5 in fp32); scores run two chunks ahead of the
    numerator so the exps overlap PE's numerator matmuls
  - O = E.T-weighted sum of cu, accumulated on PE in PSUM over chunks of
    k-tiles, then folded into an SBUF fp32 accumulator by DVE. The matmul
    reads the gathered tiles' [128, 8, 0:64] strided view (cols 64:128 of
    each 256 B row are pad)
  - rowsums r = E.T @ ones accumulated in a persistent PSUM bank
  - out = relu(O @ W) * (1/r), with the 1/r per-partition scale fused into
    the final ReLU PSUM->SBUF copy (valid since r > 0). Phase B (transpose
    on PE + W projection) runs as a batched tail after the chunk loop --
    interleaving it into the last chunk measured 130 us slower on HW

Column layout of the gathered cu is [review slots 0-7 | sec slots 0-7]
(instead of the reference's interleaved layout); the host permutes W's rows
to match, so results are identical.
"""

import os
import sys

import ml_dtypes
import numpy as np

for _p in ("/opt/trn_rl_repo", "/root/.axon_site/_ro/trn_rl_repo"):
    if os.path.isdir(_p) and _p not in sys.path:
        sys.path.append(_p)

import concourse.bass as bass  # noqa: E402
import concourse.mybir as mybir  # noqa: E402
import concourse.tile as tile  # noqa: E402
from concourse import bacc  # noqa: E402
from concourse.bass_utils import run_bass_kernel_spmd  # noqa: E402
from concourse.masks import make_identity  # noqa: E402

P = 128
D = 64
DP = 128           # gather-source row pitch (bf16 rows padded 64 -> 128 so
                   # each row is 256 B, the dma_gather stride granularity)
NK = 6000          # keys per side
NKP = 6144         # padded to 48 full k-tiles
KT = NKP // P      # 48
KT_CALC = 47       # k-tiles that carry real keys (kt 47 is all padding)
QOUT = 1500        # query rows per core (6000 / 4 cores per side)
QP = 1536          # padded to 12 full q-subtiles
NQS = QP // P      # 12
HID = 1024
NR = 30000         # review_vecs rows
NS = 6000          # secondary source rows
CHUNK_SIZES = tuple(int(x) for x in os.environ.get("K_CHUNKS", "2,4,4,4,4,4,4,4,4,4,4,4,2").split(","))
assert sum(CHUNK_SIZES) == 48
CHUNK_STARTS = tuple(int(np.cumsum((0,) + CHUNK_SIZES)[i]) for i in range(len(CHUNK_SIZES)))
F32 = mybir.dt.float32
BF16 = mybir.dt.bfloat16
I32 = mybir.dt.int32
I16 = mybir.dt.int16

AF = mybir.ActivationFunctionType


def _emit_body(nc, tc, ctx_pools, tensors):
    """Emit one full pass of the kernel body inside an open TileContext."""
    from contextlib import ExitStack

    keysT, qvT, adj_r, adj_s, src_r, src_s, w, ebias, out = tensors
    const, psum, psum_b, psum_r = ctx_pools

    # ---- persistent tiles -------------------------------------------------
    identity = const.tile([P, P], F32, tag="identity")
    make_identity(nc, identity[:])
    ones = const.tile([P, 1], BF16, tag="ones")
    nc.gpsimd.memset(ones[:], 1.0)

    # load order matters for the pipeline fill: queries + first key columns
    # first (gate the first scores), then adjacency (gates the gathers).
    # 64-partition tiles: the matmul contraction dim is just D=64, no
    # zero-padding to 128 needed.
    qvT_sb = const.tile([D, QP], BF16, tag="qvT")
    nc.sync.dma_start(qvT_sb[:, 0:512], qvT[:, 0:512])
    nc.sync.dma_start(qvT_sb[:, 512:], qvT[:, 512:])

    vecsT = const.tile([D, NKP], BF16, tag="vecsT")
    nc.sync.dma_start(vecsT[:, :NKP // 8], keysT[:, :NKP // 8])

    adj_r_sb = const.tile([P, KT, 64], I16, tag="adjr")
    nc.sync.dma_start(adj_r_sb[:], adj_r[:, :, :])
    adj_s_sb = const.tile([P, KT, 64], I16, tag="adjs")
    nc.sync.dma_start(adj_s_sb[:], adj_s[:, :, :])

    nc.sync.dma_start(vecsT[:, NKP // 8:NKP // 2], keysT[:, NKP // 8:NKP // 2])
    nc.sync.dma_start(vecsT[:, NKP // 2:], keysT[:, NKP // 2:])

    ebias_sb = const.tile([P, 1], F32, tag="ebias")
    nc.sync.dma_start(ebias_sb[:], ebias[:, :])

    o_acc = const.tile([P, NQS, HID], F32, tag="oacc")
    r_acc = const.tile([P, NQS], F32, tag="racc")
    rinv = const.tile([P, NQS], F32, tag="rinv")

    w_sb = const.tile([P, HID // P, HID], BF16, tag="w")
    nc.sync.dma_start(w_sb[:], w[:, :, :])

    chunks = [list(range(st, min(st + cs, KT_CALC)))
              for st, cs in zip(CHUNK_STARTS, CHUNK_SIZES)]
    chunks = [c for c in chunks if c]

    with ExitStack() as ctx:
        e_pool = ctx.enter_context(tc.tile_pool(name="e_pool", bufs=12))
        g_pool = ctx.enter_context(tc.tile_pool(name="g_pool", bufs=12))
        ot_pool = ctx.enter_context(tc.tile_pool(name="ot_pool", bufs=4))
        ob_pool = ctx.enter_context(tc.tile_pool(name="ob_pool", bufs=2))

        e_tiles = {}
        g_tiles = {}
        ablate = os.environ.get("K_ABLATE", "")
        if ablate == "nogather":
            gr0 = const.tile([P, 8, DP], BF16, tag="gr0")
            gs0 = const.tile([P, 8, DP], BF16, tag="gs0")
            nc.any.memzero(gr0[:])
            nc.any.memzero(gs0[:])

        def emit_gather(chunk):
            if ablate == "nogather":
                for kt in chunk:
                    g_tiles[kt] = (gr0, gs0)
                return
            # two dma_gathers per k-tile (8 review + 8 secondary neighbor
            # rows per key row): 1024 indices each, one 256 B padded source
            # row per index. Result tiles are [128, 8 slots, 128] with the
            # payload in cols 0:64 of each slot (the numerator matmul reads
            # the strided [.., 0:64] view directly).
            for kt in chunk:
                gr = g_pool.tile([P, 8, DP], BF16, tag="gr")
                nc.gpsimd.dma_gather(
                    out_ap=gr[:],
                    in_ap=src_r[:],
                    idxs_ap=adj_r_sb[:, kt, :],
                    num_idxs=1024,
                    num_idxs_reg=1024,
                    elem_size=DP,
                    queue_num=(2 * kt) % 4,
                )
                gs = g_pool.tile([P, 8, DP], BF16, tag="gs")
                nc.gpsimd.dma_gather(
                    out_ap=gs[:],
                    in_ap=src_s[:],
                    idxs_ap=adj_s_sb[:, kt, :],
                    num_idxs=1024,
                    num_idxs_reg=1024,
                    elem_size=DP,
                    queue_num=(2 * kt + 1) % 4,
                )
                g_tiles[kt] = (gr, gs)

        def score_items(chunk):
            """One (matmul+exp) emission per item; consumed a few per
            numerator iteration so the exps overlap the previous chunk's
            numerator instead of serializing before this chunk's."""
            for kt in chunk:
                e = e_pool.tile([P, QP], BF16, tag="e")
                e_tiles[kt] = e
                for i in range(QP // 512):
                    yield kt, e, i

        def emit_score(item):
            kt, e, i = item
            lhsT = vecsT[:, kt * P:(kt + 1) * P]  # [64, 128]: K=64 contraction
            s_ps = psum.tile([P, 512], F32, tag="ps")
            nc.tensor.matmul(
                s_ps[:], lhsT, qvT_sb[:, i * 512:(i + 1) * 512],
                start=True, stop=True,
            )
            # padded key rows (6000..6015) get bias -1e30 so exp()
            # forces their attention weight to exactly zero
            bias = ebias_sb[:, 0:1] if kt == KT_CALC - 1 else 0.0
            nc.scalar.activation(
                e[:, i * 512:(i + 1) * 512], s_ps[:], AF.Exp,
                bias=bias, scale=0.125,
            )

        def emit_phase_b(j):
            """Transpose o_acc[:, j] on PE, project through W, relu*1/r,
            store. Runs after the chunk loop: interleaving it into the last
            chunk's numerator measured 130 us SLOWER on HW (PE mode switches
            + DVE dependency chains), so it stays a batched tail."""
            ots = []
            for half in range(2):
                tp = psum_b.tile([P, 512], F32, tag="pb")
                for t in range(4):
                    nc.tensor.transpose(
                        tp[:, t * P:(t + 1) * P],
                        o_acc[:, j, (half * 4 + t) * P:(half * 4 + t + 1) * P],
                        identity[:],
                    )
                ot = ot_pool.tile([P, 512], BF16, tag="ot")
                nc.vector.tensor_copy(ot[:], tp[:])
                ots.append(ot)
            if ablate == "nopf":
                return
            for h in range(HID // 512):
                pf = psum_b.tile([P, 512], F32, tag="pb")
                for t in range(HID // P):
                    nc.tensor.matmul(
                        pf[:], ots[t // 4][:, (t % 4) * P:(t % 4 + 1) * P],
                        w_sb[:, t, h * 512:(h + 1) * 512],
                        start=(t == 0), stop=(t == HID // P - 1),
                    )
                if ablate == "nostore":
                    continue
                ob = ob_pool.tile([P, 512], F32, tag="ob")
                nc.scalar.activation(ob[:], pf[:], AF.Relu, scale=rinv[:, j:j + 1])
                rows = min(P, QOUT - j * P)
                if rows > 0:
                    nc.sync.dma_start(
                        out[j * P:j * P + rows, h * 512:(h + 1) * 512], ob[:rows, :],
                    )

        emit_gather(chunks[0])
        if ablate != "gatheronly":
            # scores two chunks ahead: chunk 0 and 1 up front, then chunk
            # i+2 interleaved into chunk i's numerator -- PE always has a
            # full chunk of scored keys buffered against gather jitter
            for item in score_items(chunks[0]):
                emit_score(item)
            for item in score_items(chunks[1]):
                emit_score(item)

        for ci, chunk in enumerate(chunks):
            first_chunk = ci == 0
            last_chunk = ci == len(chunks) - 1
            if not last_chunk:
                emit_gather(chunks[ci + 1])
            if ci + 2 < len(chunks):
                next_scores = score_items(chunks[ci + 2])
                per_j = -(-3 * len(chunks[ci + 2]) // NQS)
            else:
                next_scores = iter(())
                per_j = 0

            if ablate == "gatheronly":
                continue
            r_ps = psum_r.tile([P, NQS], F32, tag="rps")
            for j in range(NQS):
                p0 = psum.tile([P, 512], F32, tag="ps")
                p1 = psum.tile([P, 512], F32, tag="ps")
                for i, kt in enumerate(chunk):
                    lhsT = e_tiles[kt][:, j * P:(j + 1) * P]
                    first = i == 0
                    last = i == len(chunk) - 1
                    gr, gs = g_tiles[kt]
                    nc.tensor.matmul(p0[:], lhsT, gr[:, :, 0:D],
                                     start=first, stop=last)
                    nc.tensor.matmul(p1[:], lhsT, gs[:, :, 0:D],
                                     start=first, stop=last)
                    nc.tensor.matmul(r_ps[:, j:j + 1], lhsT, ones[:],
                                     start=first, stop=last)
                if ablate == "nofold":
                    pass
                elif first_chunk:
                    nc.vector.tensor_copy(o_acc[:, j, 0:512], p0[:])
                    nc.vector.tensor_copy(o_acc[:, j, 512:1024], p1[:])
                else:
                    nc.vector.tensor_add(o_acc[:, j, 0:512], o_acc[:, j, 0:512], p0[:])
                    nc.vector.tensor_add(o_acc[:, j, 512:1024], o_acc[:, j, 512:1024], p1[:])
                for _ in range(per_j):
                    item = next(next_scores, None)
                    if item is not None:
                        emit_score(item)

            for item in next_scores:
                emit_score(item)
            if ablate == "nofold":
                pass
            elif first_chunk:
                nc.vector.tensor_copy(r_acc[:], r_ps[:])
            else:
                nc.vector.tensor_add(r_acc[:], r_acc[:], r_ps[:])

        if ablate not in ("gatheronly", "nofold", "nophaseb"):
            nc.vector.reciprocal(rinv[:], r_acc[:])
            for j in range(NQS):
                emit_phase_b(j)


def build_program(repeat: int = 0, scratch: int | None = 32768):
    """Build + compile the SPMD program. repeat>0 wraps the body in a
    device-side For loop (for timing) and is not used for grading."""
    from contextlib import ExitStack

    kw = {} if scratch is None else dict(dynamic_dma_scratch_size=scratch)
    nc = bacc.Bacc("TRN2", target_bir_lowering=False, debug=False, num_devices=8,
                   num_swdge_queues=4, **kw)

    keysT = nc.dram_tensor("keysT", [D, NKP], BF16, kind="ExternalInput")
    qvT = nc.dram_tensor("qvT", [D, QP], BF16, kind="ExternalInput")
    adj_r = nc.dram_tensor("adj_r", [P, KT, 64], I16, kind="ExternalInput")
    adj_s = nc.dram_tensor("adj_s", [P, KT, 64], I16, kind="ExternalInput")
    src_r = nc.dram_tensor("src_r", [NR, DP], BF16, kind="ExternalInput")
    src_s = nc.dram_tensor("src_s", [NS, DP], BF16, kind="ExternalInput")
    w = nc.dram_tensor("w", [P, HID // P, HID], BF16, kind="ExternalInput")
    ebias = nc.dram_tensor("ebias", [P, 1], F32, kind="ExternalInput")
    out = nc.dram_tensor("out", [QOUT, HID], F32, kind="ExternalOutput")

    tensors = (keysT, qvT, adj_r, adj_s, src_r, src_s, w, ebias, out)

    with tile.TileContext(nc) as tc, ExitStack() as ctx:
        const = ctx.enter_context(tc.tile_pool(name="const", bufs=1))
        psum = ctx.enter_context(tc.tile_pool(name="psum", bufs=4, space="PSUM"))
        psum_b = ctx.enter_context(tc.tile_pool(name="psum_b", bufs=2, space="PSUM"))
        psum_r = ctx.enter_context(tc.tile_pool(name="psum_r", bufs=2, space="PSUM"))
        pools = (const, psum, psum_b, psum_r)
        if repeat > 1:
            # device-side loop: program size stays constant as repeat grows,
            # so repeat-marginal timing isolates true body time (static
            # unrolling confounds it with per-call program-size overheads)
            with tc.For_i(0, repeat, 1):
                _emit_body(nc, tc, pools, tensors)
        else:
            _emit_body(nc, tc, pools, tensors)

    nc.compile()
    return nc


def _permute_w(w_full: np.ndarray) -> np.ndarray:
    """Reference cu columns are slot-interleaved [r0 i0 r1 i1 ...]; the kernel
    gathers [r0..r7 | i0..i7]. Permute W rows to match, then pre-tile to
    [128, 8, 1024] for the on-device layout."""
    wr = w_full.reshape(8, 2, D, HID)
    w_perm = np.concatenate(
        [wr[:, 0].reshape(8 * D, HID), wr[:, 1].reshape(8 * D, HID)], axis=0,
    )
    return np.ascontiguousarray(
        w_perm.reshape(HID // P, P, HID).transpose(1, 0, 2),
    )


def _wrap_adj(adj: np.ndarray) -> np.ndarray:
    """[6000, 8] -> [128, KT, 64] int16 in dma_gather index layout: per
    k-tile, flat index i = slot*128 + row (so gathered row i lands at
    out[i%128, i//128]), wrapped into 16 partitions (entry n at
    [n%16, n//16]) and replicated 8x to fill 128 partitions. Padded key
    rows index row 0 (their attention weight is forced to zero by ebias)."""
    a = np.zeros((NKP, 8), dtype=np.int64)
    a[:NK] = adj
    tiles = a.reshape(KT, P, 8)                                # [kt, p, c]
    idx_flat = tiles.transpose(0, 2, 1).reshape(KT, 1024)      # i = c*128+p
    idx16 = idx_flat.reshape(KT, 64, 16).transpose(0, 2, 1)    # [kt, 16, 64]
    full = np.tile(idx16, (1, 8, 1))                           # [kt, 128, 64]
    return np.ascontiguousarray(full.transpose(1, 0, 2)).astype(np.int16)


def _host_inputs(review_vecs, user_vecs, item_vecs, user_weights, item_weights,
                 user_review_adj, user_item_adj, item_review_adj, item_user_adj):
    review_vecs = np.asarray(review_vecs, dtype=np.float32)
    user_vecs = np.asarray(user_vecs, dtype=np.float32)
    item_vecs = np.asarray(item_vecs, dtype=np.float32)

    def _pad_src(v: np.ndarray) -> np.ndarray:
        out = np.zeros((v.shape[0], DP), dtype=ml_dtypes.bfloat16)
        out[:, :D] = v.astype(ml_dtypes.bfloat16)
        return out

    src_r_pad = _pad_src(review_vecs)
    sides = {}
    for side, keys, adj_r, adj_s, src_s, w_full in (
        ("user", user_vecs, user_review_adj, user_item_adj, item_vecs, user_weights),
        ("item", item_vecs, item_review_adj, item_user_adj, user_vecs, item_weights),
    ):
        keysT = np.zeros((D, NKP), dtype=ml_dtypes.bfloat16)
        keysT[:, :NK] = keys.T.astype(ml_dtypes.bfloat16)
        sides[side] = dict(
            keysT=keysT,
            adj_r=_wrap_adj(np.asarray(adj_r, dtype=np.int64)),
            adj_s=_wrap_adj(np.asarray(adj_s, dtype=np.int64)),
            src_s=_pad_src(np.asarray(src_s, dtype=np.float32)),
            w=_permute_w(np.asarray(w_full, dtype=np.float32)),
            keys=keys,
        )

    ebias = np.zeros((P, 1), dtype=np.float32)
    ebias[NK - (KT_CALC - 1) * P:] = -1e30

    in_maps = []
    for c in range(8):
        s = sides["user" if c < 4 else "item"]
        b = c % 4
        qv = s["keys"][b * QOUT:(b + 1) * QOUT]  # [1500, 64]
        qvT = np.empty((D, QP), dtype=np.float32)
        qvT[:, :QOUT] = qv.T
        qvT[:, QOUT:] = qv.T[:, :QP - QOUT]  # pad with real vectors (finite rowsums)
        in_maps.append(dict(
            keysT=s["keysT"], qvT=np.ascontiguousarray(qvT.astype(ml_dtypes.bfloat16)),
            adj_r=s["adj_r"], adj_s=s["adj_s"],
            src_r=src_r_pad, src_s=s["src_s"],
            w=s["w"].astype(ml_dtypes.bfloat16), ebias=ebias,
        ))
    return in_maps


_NC_CACHE = None


def kernel(**inputs):
    global _NC_CACHE
    if _NC_CACHE is None:
        _NC_CACHE = build_program()
    nc = _NC_CACHE
    in_maps = _host_inputs(**inputs)
    res = run_bass_kernel_spmd(nc, in_maps, core_ids=list(range(8)))
    outs = [res.results[c]["out"] for c in range(8)]
    user_output = np.concatenate(outs[0:4], axis=0)
    item_output = np.concatenate(outs[4:8], axis=0)
    return user_output, item_output
